# revision 2
# baseline (speedup 1.0000x reference)
"""AtomConv (GCN message passing) distributed Bass kernel for 8 TRN2 NeuronCores.

out = relu(D^-1/2 (A+I) D^-1/2 (atom @ W.T + b)),  A = 3.2M random edges over 100K nodes.

Sharding (per the dst-routing hint): nodes 12500/core, edges routed to the core
owning the destination, weights replicated. Aggregation runs in 6-dim input
space: z[r] = [atom[r]*dis[r], dis[r]]; agg[c] = sum_{r->c} z[r];
out[c] = relu((dis[c]*agg[c]) @ [W|b].T).

Device mechanism: the per-edge gather of z rows uses gpsimd dma_gather
(256B elements, int16 indices) from 4 src-quarter tables on 4 SWDGE queues.
Slot grids are degree-sorted per (core, quarter) so grid columns are dense;
grid shapes are cross-core-maxed templates so one SPMD graph serves all 8
cores. Quarter partials are re-aligned via 3 more gathers, then a DVE matvec
(6->16) + relu finishes on device. Host work is routing/layout only
(bincount, sort, index packing) plus the final row unpermute/concat.
"""

import os
import numpy as np

N_NODES = 100000
N_IN = 5
N_OUT = 16
N_CORES = 8
NPC = N_NODES // N_CORES            # 12500
NQ = 4
QSZ = N_NODES // NQ                 # 25000 (int16-safe index range)
ES = 64                             # gather element = 64 f32 = 256B
P = 128
NPC_PAD = ((NPC + P - 1) // P) * P  # 12544
CHUNKS = NPC_PAD // P               # 98
QROWS = QSZ + 200                   # per-quarter z-table rows (row 0 = zeros)
AW = (QSZ + P - 1) // P             # 196 atom-wrap cols
MAX_CALL = 8192                     # slots per dma_gather call

LAST_EXEC_NS = None


def _host_prepare(atom, edge_index, W, b):
    src = np.asarray(edge_index[0]).astype(np.int64)
    dst = np.asarray(edge_index[1]).astype(np.int64)
    # deg includes the self loop; self-loop messages are added directly on
    # device (no gather slot needed)
    deg = (np.bincount(dst, minlength=N_NODES) + 1.0).astype(np.float32)

    core_of = dst // NPC
    quarter = src // QSZ

    per = {}
    dq = np.zeros((N_CORES, NQ, NPC_PAD), np.int64)
    for ci in range(N_CORES):
        mc = core_of == ci
        s_c, d_c, q_c = src[mc], dst[mc] - ci * NPC, quarter[mc]
        for q in range(NQ):
            mq = q_c == q
            d_loc = d_c[mq]
            s_loc = s_c[mq] - q * QSZ
            per[(ci, q)] = (d_loc, s_loc)
            dq[ci, q, :NPC] = np.bincount(d_loc, minlength=NPC)

    pi = np.argsort(dq, axis=2, kind="stable")          # ascending degree
    dq_sorted = np.take_along_axis(dq, pi, axis=2)
    K = dq_sorted.reshape(N_CORES, NQ, CHUNKS, P).max(axis=3).max(axis=0)
    K = np.maximum(K, 1).astype(np.int64)               # [NQ, CHUNKS] template

    idx_feeds = []
    for ci in range(N_CORES):
        q_feeds = []
        for q in range(NQ):
            d_loc, s_loc = per[(ci, q)]
            counts = dq[ci, q, :NPC]
            Kq = int(K[q].max())
            mat = np.zeros((NPC_PAD, Kq), np.int16)
            if len(d_loc):
                order = np.argsort(d_loc, kind="stable")
                d_sorted, s_sorted = d_loc[order], s_loc[order]
                starts = np.zeros(NPC, np.int64)
                starts[1:] = np.cumsum(counts)[:-1]
                kk = np.arange(len(d_sorted)) - starts[d_sorted]
                mat[d_sorted, kk] = (s_sorted + 1).astype(np.int16)
            g = mat[pi[ci, q]]
            slots = [
                g[c * P:(c + 1) * P, : int(K[q, c])].T.reshape(-1)
                for c in range(CHUNKS)
            ]
            q_feeds.append(np.concatenate(slots))
        idx_feeds.append(q_feeds)

    comb_feeds = []
    for ci in range(N_CORES):
        inv = np.empty((NQ, NPC_PAD), np.int64)
        for q in range(NQ):
            inv[q, pi[ci, q]] = np.arange(NPC_PAD)
        comb_feeds.append([inv[q][pi[ci, 0]].astype(np.int16) for q in range(1, NQ)])

    deg_pi0 = []
    atom_pi0 = []
    for ci in range(N_CORES):
        dpc = np.zeros(NPC_PAD, np.float32)
        dpc[:NPC] = deg[ci * NPC:(ci + 1) * NPC]
        dpc = np.maximum(dpc[pi[ci, 0]], 1.0)
        deg_pi0.append(np.ascontiguousarray(dpc.reshape(CHUNKS, P).T))  # [P, CH]
        apc = np.zeros((NPC_PAD, N_IN), np.float32)
        apc[:NPC] = np.asarray(atom, np.float32)[ci * NPC:(ci + 1) * NPC]
        apc = apc[pi[ci, 0]].reshape(CHUNKS, P, N_IN)
        atom_pi0.append(np.ascontiguousarray(apc.transpose(1, 0, 2)))  # [P, CH, 5]

    a = np.asarray(atom, np.float32)
    atom_q = np.zeros((NQ, P, AW, N_IN), np.float32)
    deg_q = np.ones((NQ, P, AW), np.float32)
    for q in range(NQ):
        blk = np.zeros((P * AW, N_IN), np.float32)
        blk[:QSZ] = a[q * QSZ:(q + 1) * QSZ]
        atom_q[q] = blk.reshape(P, AW, N_IN)
        dblk = np.ones(P * AW, np.float32)
        dblk[:QSZ] = np.maximum(deg[q * QSZ:(q + 1) * QSZ], 1.0)
        deg_q[q] = dblk.reshape(P, AW)

    W_ext = np.zeros((N_OUT, 6), np.float64)
    W_ext[:, :5] = np.asarray(W, np.float64)
    W_ext[:, 5] = np.asarray(b, np.float64)

    return dict(K=K, pi=pi, idx_feeds=idx_feeds, comb_feeds=comb_feeds,
                deg_pi0=deg_pi0, atom_pi0=atom_pi0, atom_q=atom_q, deg_q=deg_q,
                W_ext=W_ext)


def _wrap16(flat):
    """idx j -> sbuf (j%16, j//16), replicated across the 8 q7 cores."""
    n = len(flat)
    w = flat.reshape(n // 16, 16).T
    return np.ascontiguousarray(np.tile(w, (8, 1)).astype(np.int16))


def _plan_calls(K):
    plans = []
    for q in range(NQ):
        calls, cur, cur_slots, off = [], [], 0, 0
        for c in range(CHUNKS):
            s = int(K[q, c]) * P
            if cur_slots + s > MAX_CALL and cur:
                calls.append((off, cur_slots, cur))
                off += cur_slots
                cur, cur_slots = [], 0
            cur.append((c, int(K[q, c]), cur_slots))
            cur_slots += s
        if cur:
            calls.append((off, cur_slots, cur))
        plans.append(calls)
    return plans


def _build_graph(K, W_ext):
    import concourse.bass as bass
    import concourse.bacc as bacc
    import concourse.mybir as mybir
    import concourse.tile as tile
    from concourse import library_config

    f32 = mybir.dt.float32
    i16 = mybir.dt.int16
    AT = mybir.AluOpType
    AX = mybir.AxisListType

    plans = _plan_calls(K)
    S_q = [sum(int(K[q, c]) * P for c in range(CHUNKS)) for q in range(NQ)]

    nc = bacc.Bacc("TRN2", target_bir_lowering=False, debug=False,
                   num_swdge_queues=4)

    atom_in = nc.dram_tensor("atom_q", [NQ, P, AW, N_IN], f32, kind="ExternalInput")
    degq_in = nc.dram_tensor("deg_q", [NQ, P, AW], f32, kind="ExternalInput")
    degp_in = nc.dram_tensor("deg_pi0", [P, CHUNKS], f32, kind="ExternalInput")
    atomp_in = nc.dram_tensor("atom_pi0", [P, CHUNKS, N_IN], f32, kind="ExternalInput")
    wrep_in = nc.dram_tensor("w_rep", [P, 6 * N_OUT], f32, kind="ExternalInput")
    idx_ins = [nc.dram_tensor(f"idx_q{q}", [P, S_q[q] // 16], i16, kind="ExternalInput")
               for q in range(NQ)]
    comb_ins = [nc.dram_tensor(f"comb_q{q}", [P, NPC_PAD // 16], i16, kind="ExternalInput")
                for q in range(1, NQ)]
    out_t = nc.dram_tensor("out", [NPC_PAD, N_OUT], f32, kind="ExternalOutput")

    z_dram = nc.dram_tensor("z_tab", [NQ, QROWS, ES], f32, kind="Internal")
    part_dram = nc.dram_tensor("part", [NQ - 1, NPC_PAD, ES], f32, kind="Internal")

    with tile.TileContext(nc) as tc:
        with tc.tile_pool(name="sb", bufs=1) as pool, \
             tc.tile_pool(name="gp", bufs=4) as gpool, \
             tc.tile_pool(name="ip", bufs=4) as ipool, \
             tc.tile_pool(name="cp", bufs=2) as cpool, \
             tc.tile_pool(name="aq", bufs=2) as aqpool:
            nc.gpsimd.load_library(library_config.mlp)

            # ---- z tables: rows 1+p*AW+c <- [atom*dis | dis], row 0 <- 0
            zero64 = pool.tile([1, ES], f32)
            nc.vector.memset(zero64[:], 0.0)
            for q in range(NQ):
                nc.sync.dma_start(out=z_dram[q, 0:1, :], in_=zero64[:])
            G = 4
            GW = AW // G  # 49 rows per column-group
            for q in [1, 2, 3, 0]:
                at = pool.tile([P, AW * N_IN], f32, tag="at")
                dg = pool.tile([P, AW], f32, tag="dg")
                ds = pool.tile([P, AW], f32, tag="ds")
                nc.sync.dma_start(out=at[:], in_=atom_in[q].rearrange("p a f -> p (a f)"))
                nc.sync.dma_start(out=dg[:], in_=degq_in[q])
                nc.vector.reciprocal(ds[:], dg[:])
                nc.scalar.activation(ds[:], ds[:], mybir.ActivationFunctionType.Sqrt)
                atv = at[:].rearrange("p (a f) -> p a f", f=N_IN)
                for g in range(G):
                    # 64-wide staging tile: cols 0:6 real, 6:64 garbage (never
                    # read back -- the gather extract only touches cols 0:6)
                    zb = pool.tile([P, GW * ES], f32, tag="zb64")
                    zbv = zb[:].rearrange("p (a e) -> p a e", e=ES)
                    sl = slice(g * GW, (g + 1) * GW)
                    for f in range(N_IN):
                        nc.vector.tensor_tensor(zbv[:, :, f], atv[:, sl, f], ds[:, sl], op=AT.mult)
                    nc.vector.tensor_copy(zbv[:, :, 5], ds[:, sl])
                    nc.sync.dma_start(
                        out=bass.AP(z_dram, q * QROWS * ES + (1 + g * GW) * ES,
                                    [[AW * ES, P], [1, GW * ES]]),
                        in_=zb[:],
                    )

            acc = pool.tile([P, CHUNKS * 6], f32)
            accv = acc[:].rearrange("p (c f) -> p c f", f=6)
            nc.vector.memset(acc[:], 0.0)

            # dis_dst and self-loop term have no gather deps: compute early
            dgp = pool.tile([P, CHUNKS], f32)
            dsp = pool.tile([P, CHUNKS], f32)
            nc.sync.dma_start(out=dgp[:], in_=degp_in.ap())
            nc.vector.reciprocal(dsp[:], dgp[:])
            nc.scalar.activation(dsp[:], dsp[:], mybir.ActivationFunctionType.Sqrt)
            sl6 = pool.tile([P, CHUNKS * 6], f32)
            sl6v = sl6[:].rearrange("p (c f) -> p c f", f=6)
            ap0 = pool.tile([P, CHUNKS * N_IN], f32, tag="ap0")
            nc.sync.dma_start(out=ap0[:], in_=atomp_in.ap().rearrange("p c f -> p (c f)"))
            ap0v = ap0[:].rearrange("p (c f) -> p c f", f=N_IN)
            for f in range(N_IN):
                nc.vector.tensor_tensor(sl6v[:, :, f], ap0v[:, :, f], dsp[:], op=AT.mult)
            nc.vector.tensor_copy(sl6v[:, :, 5], dsp[:])
            accq = None

            qn = 0
            for q in [1, 2, 3, 0]:
                if q > 0:
                    accq = aqpool.tile([P, CHUNKS * 6], f32, tag="accq")
                tgt = accv if q == 0 else accq[:].rearrange("p (c f) -> p c f", f=6)
                for (off, nslots, chunks) in plans[q]:
                    it = ipool.tile([P, MAX_CALL // 16], i16, tag="idx")
                    nc.sync.dma_start(
                        out=it[:, : nslots // 16],
                        in_=idx_ins[q][:, off // 16:(off + nslots) // 16])
                    gb = gpool.tile([P, (MAX_CALL // P) * ES], f32, tag="gb")
                    gbv = gb[:].rearrange("p (m d) -> p m d", m=MAX_CALL // P)
                    nc.gpsimd.dma_gather(
                        out_ap=gbv[:, : nslots // P, :],
                        in_ap=z_dram[q],
                        idxs_ap=it[:, : nslots // 16],
                        num_idxs=nslots,
                        num_idxs_reg=nslots,
                        elem_size=ES,
                        single_packet=False,
                        queue_num=qn % 4,
                    )
                    qn += 1
                    for (c, kc, loff) in chunks:
                        col0 = loff // P
                        src = gbv[:, col0:col0 + kc, 0:6].rearrange("p m d -> p d m")
                        if q == 0:
                            t6 = gpool.tile([P, 6], f32, tag="t6")
                            nc.vector.tensor_reduce(t6[:], src, axis=AX.X, op=AT.add)
                            nc.vector.tensor_tensor(tgt[:, c, :], tgt[:, c, :], t6[:], op=AT.add)
                        else:
                            nc.vector.tensor_reduce(tgt[:, c, :], src, axis=AX.X, op=AT.add)
                if q > 0:
                    # stage pi_q-ordered partial to DRAM rows r=c*128+p, cols 0:6
                    nc.sync.dma_start(
                        out=bass.AP(part_dram, (q - 1) * NPC_PAD * ES,
                                    [[ES, P], [P * ES, CHUNKS], [1, 6]]),
                        in_=tgt,
                    )
                    # combine: gather this quarter's partial into pi_0 order
                    ct = ipool.tile([P, NPC_PAD // 16], i16, tag="cidx")
                    nc.sync.dma_start(out=ct[:], in_=comb_ins[q - 1].ap())
                    gc = cpool.tile([P, CHUNKS * ES], f32, tag="gc")
                    gcv = gc[:].rearrange("p (m d) -> p m d", m=CHUNKS)
                    nc.gpsimd.dma_gather(
                        out_ap=gcv,
                        in_ap=part_dram[q - 1],
                        idxs_ap=ct[:],
                        num_idxs=NPC_PAD,
                        num_idxs_reg=NPC_PAD,
                        elem_size=ES,
                        single_packet=False,
                        queue_num=(qn + 2) % 4,
                    )
                    nc.vector.tensor_tensor(accv, accv, gcv[:, :, 0:6], op=AT.add)

            # ---- finish: add self-loop term, dis_dst scale, 6->16 matvec, relu
            nc.vector.tensor_tensor(acc[:], acc[:], sl6[:], op=AT.add)
            for f in range(6):
                nc.vector.tensor_tensor(accv[:, :, f], accv[:, :, f], dsp[:], op=AT.mult)

            wr = pool.tile([P, 6 * N_OUT], f32, tag="wr")
            nc.sync.dma_start(out=wr[:], in_=wrep_in.ap())
            wrv = wr[:].rearrange("p (f o) -> p f o", o=N_OUT)
            o16 = pool.tile([P, CHUNKS * N_OUT], f32)
            o16v = o16[:].rearrange("p (c o) -> p c o", o=N_OUT)
            t16 = pool.tile([P, CHUNKS * N_OUT], f32, tag="t16")
            t16v = t16[:].rearrange("p (c o) -> p c o", o=N_OUT)
            for f in range(6):
                a_b = accv[:, :, f:f + 1].to_broadcast([P, CHUNKS, N_OUT])
                w_b = wrv[:, f:f + 1, :].to_broadcast([P, CHUNKS, N_OUT])
                if f == 0:
                    nc.vector.tensor_tensor(o16v, a_b, w_b, op=AT.mult)
                else:
                    nc.vector.tensor_tensor(t16v, a_b, w_b, op=AT.mult)
                    nc.vector.tensor_tensor(o16v, o16v, t16v, op=AT.add)
            nc.vector.tensor_scalar_max(o16[:], o16[:], 0.0)

            # out rows r=c*128+p
            nc.sync.dma_start(
                out=bass.AP(out_t, 0, [[N_OUT, P], [P * N_OUT, CHUNKS], [1, N_OUT]]),
                in_=o16v,
            )

    nc.compile()
    return nc


def kernel(**inputs):
    global LAST_EXEC_NS
    atom = inputs["atom"]
    edge_index = inputs["edge_index"]
    W = inputs["W"]
    b = inputs["b"]

    prep = _host_prepare(atom, edge_index, W, b)
    nc = _build_graph(prep["K"], prep["W_ext"])

    from concourse import bass_utils

    in_maps = []
    for ci in range(N_CORES):
        m = {
            "atom_q": prep["atom_q"],
            "deg_q": prep["deg_q"],
            "deg_pi0": prep["deg_pi0"][ci],
            "atom_pi0": prep["atom_pi0"][ci],
            "w_rep": np.ascontiguousarray(
                np.tile(prep["W_ext"].astype(np.float32).T.reshape(1, 6 * N_OUT),
                        (P, 1))),
            "out": np.zeros((NPC_PAD, N_OUT), np.float32),
        }
        for q in range(NQ):
            m[f"idx_q{q}"] = _wrap16(prep["idx_feeds"][ci][q])
        for q in range(1, NQ):
            m[f"comb_q{q}"] = _wrap16(prep["comb_feeds"][ci][q - 1])
        m.pop("out")
        in_maps.append(m)

    trace = bool(os.environ.get("KERNEL_TRACE"))
    if trace:
        try:
            import tracing_shim
            tracing_shim.install()
        except Exception:
            trace = False

    res = bass_utils.run_bass_kernel_spmd(
        nc, in_maps, core_ids=list(range(N_CORES)), trace=trace
    )
    LAST_EXEC_NS = res.exec_time_ns
    globals()["LAST_RES"] = res

    out = np.empty((N_NODES, N_OUT), np.float32)
    for ci in range(N_CORES):
        rows = res.results[ci]["out"]  # [NPC_PAD, 16], row j -> node pi0[j]
        pi0 = prep["pi"][ci, 0]
        real = pi0 < NPC
        out[ci * NPC + pi0[real]] = rows[real]
    return out



# revision 3
# speedup vs baseline: 1.1310x; 1.1310x over previous
"""AtomConv (GCN message passing) distributed Bass kernel for 8 TRN2 NeuronCores.

out = relu(D^-1/2 (A+I) D^-1/2 (atom @ W.T + b)),  A = 3.2M random edges over 100K nodes.

Sharding (per the dst-routing hint): nodes 12500/core, edges routed to the core
owning the destination, weights replicated. Aggregation runs in 6-dim input
space: z[s] = [atom[s]*dis[s], dis[s]]; agg[d] = sum_{s->d} z[s];
out[d] = relu((dis[d]*agg[d]) @ [W|b].T).

Device mechanism: one global z-table [25088 rows x 256B] where row k packs the
z-vectors of nodes 4k..4k+3 at 6-f32 pitch (cols 24:64 zero).  The per-edge
gather uses gpsimd dma_gather (256B elements, int16 row idx = src//4); a
host-fed one-hot mask [slot, 4] selects the wanted sub-row on the DVE
(mask 0 for padding slots, so no zero-row/idx+1 tricks are needed).  Slots
form a single degree-sorted grid (128 dst rows/chunk x K[c] cols, K maxed
across cores so one SPMD graph serves all 8 cores); each chunk's masked slots
reduce directly into the accumulator (no cross-quarter combines).  A DVE
matvec (6->16) + relu finishes on device.  Host work is routing/layout only
(bincount, sort, index/mask packing) plus the final row unpermute/concat.
"""

import os
import numpy as np

N_NODES = 100000
N_IN = 5
N_OUT = 16
N_CORES = 8
NPC = N_NODES // N_CORES            # 12500
P = 128
NPC_PAD = ((NPC + P - 1) // P) * P  # 12544
CHUNKS = NPC_PAD // P               # 98
RPN = 4                             # nodes packed per 256B table row
TA = 196                            # table "a" dim: rows k = a*128+p
TROWS = TA * P                      # 25088 table rows >= 100000/4
ES = 64                             # table row = 64 f32 = 256B
MAX_CALL = 8192                     # slots per dma_gather call

LAST_EXEC_NS = None


def _host_prepare(atom, edge_index, W, b):
    src = np.asarray(edge_index[0]).astype(np.int64)
    dst = np.asarray(edge_index[1]).astype(np.int64)
    # deg includes the self loop; self-loop messages are added directly on
    # device (no gather slot needed)
    deg = (np.bincount(dst, minlength=N_NODES) + 1.0).astype(np.float32)

    core_of = dst // NPC

    # per-core in-core dst degree (real edges only) -> degree-sorted grid
    cnt = np.zeros((N_CORES, NPC_PAD), np.int64)
    per = {}
    for ci in range(N_CORES):
        mc = core_of == ci
        per[ci] = (dst[mc] - ci * NPC, src[mc])
        cnt[ci, :NPC] = np.bincount(per[ci][0], minlength=NPC)
    pi = np.argsort(cnt, axis=1, kind="stable")          # ascending degree
    cnt_sorted = np.take_along_axis(cnt, pi, axis=1)
    K = cnt_sorted.reshape(N_CORES, CHUNKS, P).max(axis=2).max(axis=0)
    K = np.maximum(K, 1).astype(np.int64)                # [CHUNKS] template

    # call plan: whole chunks greedily packed into <= MAX_CALL slots
    calls, cur, cur_slots = [], [], 0
    for c in range(CHUNKS):
        s = int(K[c]) * P
        if cur_slots + s > MAX_CALL and cur:
            calls.append(cur)
            cur, cur_slots = [], 0
        cur.append((c, int(K[c]), cur_slots // P))       # (chunk, K, col offset)
        cur_slots += s
    if cur:
        calls.append(cur)
    S_TOT = int(K.sum()) * P
    M_TOT = S_TOT // P

    Kmax = int(K.max())
    idx_feeds, mask_feeds = [], []
    for ci in range(N_CORES):
        d_loc, s_glob = per[ci]
        order = np.argsort(d_loc, kind="stable")
        d_s, s_s = d_loc[order], s_glob[order]
        starts = np.zeros(NPC, np.int64)
        starts[1:] = np.cumsum(cnt[ci, :NPC])[:-1]
        kk = np.arange(len(d_s)) - starts[d_s]
        mat_idx = np.zeros((NPC_PAD, Kmax), np.int16)
        mat_sub = np.zeros((NPC_PAD, Kmax), np.int8)
        mat_val = np.zeros((NPC_PAD, Kmax), bool)
        mat_idx[d_s, kk] = (s_s // RPN).astype(np.int16)
        mat_sub[d_s, kk] = (s_s % RPN).astype(np.int8)
        mat_val[d_s, kk] = True
        g_idx = mat_idx[pi[ci]]
        g_sub = mat_sub[pi[ci]]
        g_val = mat_val[pi[ci]]
        idx_parts, mask_parts = [], []
        for call in calls:
            for (c, kc, _) in call:
                rows = slice(c * P, (c + 1) * P)
                idx_parts.append(g_idx[rows, :kc].T.reshape(-1))
                sub = g_sub[rows, :kc].T.reshape(-1)
                val = g_val[rows, :kc].T.reshape(-1)
                m = np.zeros((len(sub), RPN), np.float32)
                m[np.arange(len(sub)), sub] = val.astype(np.float32)
                mask_parts.append(m)
        idx_feeds.append(np.concatenate(idx_parts))
        mflat = np.concatenate(mask_parts)               # [S_TOT, 4] slot-major
        mask_feeds.append(np.ascontiguousarray(
            mflat.reshape(M_TOT, P, RPN).transpose(1, 0, 2).reshape(P, M_TOT * RPN)))

    # atom packed in table layout: node n = 4*(a*128+p)+j at (p, a, j)
    nid = (RPN * (np.arange(TA)[None, :, None] * P + np.arange(P)[:, None, None])
           + np.arange(RPN)[None, None, :])              # [P, TA, RPN]
    valid = nid < N_NODES
    nsafe = np.where(valid, nid, 0)
    a_np = np.asarray(atom, np.float32)
    atom_pack = np.zeros((P, TA, RPN, 6), np.float32)
    atom_pack[:, :, :, :N_IN] = a_np[nsafe] * valid[..., None]
    atom_pack[:, :, :, N_IN] = 1.0
    degn = np.where(valid, deg[nsafe], 1.0).astype(np.float32)  # [P, TA, RPN]

    # dst-side (pi-ordered) feeds: row r=c*128+p -> node pi[r]
    deg_pi, atom_pi = [], []
    for ci in range(N_CORES):
        dpc = np.ones(NPC_PAD, np.float32)
        dpc[:NPC] = deg[ci * NPC:(ci + 1) * NPC]
        dpc = np.maximum(dpc[pi[ci]], 1.0)
        deg_pi.append(np.ascontiguousarray(dpc.reshape(CHUNKS, P).T))  # [P, CH]
        apc = np.zeros((NPC_PAD, 6), np.float32)
        apc[:NPC, :N_IN] = a_np[ci * NPC:(ci + 1) * NPC]
        apc[:, N_IN] = 1.0
        apc = apc[pi[ci]].reshape(CHUNKS, P, 6)
        atom_pi.append(np.ascontiguousarray(apc.transpose(1, 0, 2)))   # [P, CH, 6]

    W_ext = np.zeros((N_OUT, 6), np.float32)
    W_ext[:, :N_IN] = np.asarray(W, np.float32)
    W_ext[:, N_IN] = np.asarray(b, np.float32)

    return dict(K=K, pi=pi, calls=calls, S_TOT=S_TOT, M_TOT=M_TOT,
                idx_feeds=idx_feeds, mask_feeds=mask_feeds,
                atom_pack=atom_pack.reshape(P, TA * RPN * 6), degn=degn.reshape(P, TA * RPN),
                deg_pi=deg_pi, atom_pi=atom_pi, W_ext=W_ext)


def _wrap16(flat):
    """idx j -> sbuf (j%16, j//16), replicated across the 8 q7 cores."""
    n = len(flat)
    w = flat.reshape(n // 16, 16).T
    return np.ascontiguousarray(np.tile(w, (8, 1)).astype(np.int16))


def _build_graph(K, calls, S_TOT, M_TOT):
    import concourse.bass as bass
    import concourse.bacc as bacc
    import concourse.mybir as mybir
    import concourse.tile as tile
    from concourse import library_config

    f32 = mybir.dt.float32
    i16 = mybir.dt.int16
    AT = mybir.AluOpType
    AX = mybir.AxisListType

    S_call_max = max(sum(kc for (_, kc, _) in call) for call in calls) * P
    M_call_max = S_call_max // P

    nc = bacc.Bacc("TRN2", target_bir_lowering=False, debug=False,
                   num_swdge_queues=4)

    atom_in = nc.dram_tensor("atom_pack", [P, TA * RPN * 6], f32, kind="ExternalInput")
    degn_in = nc.dram_tensor("degn", [P, TA * RPN], f32, kind="ExternalInput")
    degp_in = nc.dram_tensor("deg_pi", [P, CHUNKS], f32, kind="ExternalInput")
    atomp_in = nc.dram_tensor("atom_pi", [P, CHUNKS, 6], f32, kind="ExternalInput")
    wrep_in = nc.dram_tensor("w_rep", [P, 6 * N_OUT], f32, kind="ExternalInput")
    idx_in = nc.dram_tensor("idx_all", [P, S_TOT // 16], i16, kind="ExternalInput")
    mask_in = nc.dram_tensor("mask_all", [P, M_TOT * RPN], f32, kind="ExternalInput")
    out_t = nc.dram_tensor("out", [NPC_PAD, N_OUT], f32, kind="ExternalOutput")

    z_dram = nc.dram_tensor("z_tab", [TROWS, ES], f32, kind="Internal")

    with tile.TileContext(nc) as tc:
        with tc.tile_pool(name="sb", bufs=1) as pool, \
             tc.tile_pool(name="zt", bufs=2) as ztpool, \
             tc.tile_pool(name="gp", bufs=3) as gpool, \
             tc.tile_pool(name="tp", bufs=2) as tpool, \
             tc.tile_pool(name="ip", bufs=3) as ipool, \
             tc.tile_pool(name="mp", bufs=3) as mpool:
            nc.gpsimd.load_library(library_config.mlp)

            # ---- source-side dis and packed z-table build (2 pieces)
            at = pool.tile([P, TA * RPN * 6], f32, tag="at")
            dgn = pool.tile([P, TA * RPN], f32, tag="dgn")
            dsn = pool.tile([P, TA * RPN], f32, tag="dsn")
            nc.sync.dma_start(out=at[:], in_=atom_in.ap())
            nc.sync.dma_start(out=dgn[:], in_=degn_in.ap())
            nc.vector.reciprocal(dsn[:], dgn[:])
            nc.scalar.activation(dsn[:], dsn[:], mybir.ActivationFunctionType.Sqrt)

            atv = at[:].rearrange("p (a j f) -> p a j f", j=RPN, f=6)
            dsv = dsn[:].rearrange("p (a j f) -> p a j f", j=RPN, f=1)
            HA = TA // 2
            for piece in range(2):
                zt = ztpool.tile([P, HA * ES], f32, tag="zt")
                nc.vector.memset(zt[:], 0.0)
                ztv = zt[:].rearrange("p (a e) -> p a e", e=ES)
                ztj = ztv[:, :, 0:RPN * 6].rearrange("p a (j f) -> p a j f", f=6)
                sl = slice(piece * HA, (piece + 1) * HA)
                nc.vector.tensor_tensor(
                    ztj, atv[:, sl], dsv[:, sl].to_broadcast([P, HA, RPN, 6]),
                    op=AT.mult)
                nc.sync.dma_start(
                    out=bass.AP(z_dram, piece * HA * P * ES,
                                [[ES, P], [P * ES, HA], [1, ES]]),
                    in_=zt[:],
                )

            # ---- dst-side dis and self-loop term (no gather deps)
            dgp = pool.tile([P, CHUNKS], f32, tag="dgp")
            dsp = pool.tile([P, CHUNKS], f32, tag="dsp")
            nc.sync.dma_start(out=dgp[:], in_=degp_in.ap())
            nc.vector.reciprocal(dsp[:], dgp[:])
            nc.scalar.activation(dsp[:], dsp[:], mybir.ActivationFunctionType.Sqrt)
            ap0 = pool.tile([P, CHUNKS * 6], f32, tag="ap0")
            nc.sync.dma_start(out=ap0[:], in_=atomp_in.ap().rearrange("p c f -> p (c f)"))
            sl6 = pool.tile([P, CHUNKS * 6], f32, tag="sl6")
            sl6v = sl6[:].rearrange("p (c f) -> p c f", f=6)
            dspv = dsp[:].rearrange("p (c f) -> p c f", f=1)
            nc.vector.tensor_tensor(
                sl6v, ap0[:].rearrange("p (c f) -> p c f", f=6),
                dspv.to_broadcast([P, CHUNKS, 6]), op=AT.mult)

            acc = pool.tile([P, CHUNKS * 6], f32)
            accv = acc[:].rearrange("p (c f) -> p c f", f=6)

            # ---- gather + masked reduce per chunk
            qn = 0
            off = 0
            for call in calls:
                S = sum(kc for (_, kc, _) in call) * P
                M = S // P
                it = ipool.tile([P, S_call_max // 16], i16, tag="idx")
                nc.sync.dma_start(
                    out=it[:, : S // 16],
                    in_=idx_in[:, off // 16:(off + S) // 16])
                mt = mpool.tile([P, M_call_max * RPN], f32, tag="msk")
                nc.sync.dma_start(
                    out=mt[:, : M * RPN],
                    in_=mask_in[:, (off // P) * RPN:(off // P + M) * RPN])
                gb = gpool.tile([P, M_call_max * ES], f32, tag="gb")
                gbv = gb[:].rearrange("p (m e) -> p m e", m=M_call_max)
                nc.gpsimd.dma_gather(
                    out_ap=gbv[:, :M, :],
                    in_ap=z_dram.ap(),
                    idxs_ap=it[:, : S // 16],
                    num_idxs=S,
                    num_idxs_reg=S,
                    elem_size=ES,
                    single_packet=False,
                    queue_num=qn % 4,
                )
                qn += 1
                gb24 = gbv[:, :M, 0:RPN * 6].rearrange("p m (j f) -> p m j f", f=6)
                tmp = tpool.tile([P, M_call_max * RPN * 6], f32, tag="tmp")
                tmpv = tmp[:, : M * RPN * 6].rearrange("p (m j f) -> p m j f", j=RPN, f=6)
                mtv = mt[:, : M * RPN].rearrange("p (m j f) -> p m j f", j=RPN, f=1)
                nc.vector.tensor_tensor(
                    tmpv, gb24, mtv.to_broadcast([P, M, RPN, 6]), op=AT.mult)
                for (c, kc, colofs) in call:
                    seg = tmp[:, colofs * RPN * 6:(colofs + kc) * RPN * 6] \
                        .rearrange("p (k f) -> p f k", f=6)
                    nc.vector.tensor_reduce(accv[:, c, :], seg, axis=AX.X, op=AT.add)
                off += S

            # ---- finish: add self-loop term, dis_dst scale, 6->16 matvec, relu
            nc.vector.tensor_tensor(acc[:], acc[:], sl6[:], op=AT.add)
            nc.vector.tensor_tensor(
                accv, accv, dspv.to_broadcast([P, CHUNKS, 6]), op=AT.mult)

            wr = pool.tile([P, 6 * N_OUT], f32, tag="wr")
            nc.sync.dma_start(out=wr[:], in_=wrep_in.ap())
            wrv = wr[:].rearrange("p (f o) -> p f o", o=N_OUT)
            o16 = pool.tile([P, CHUNKS * N_OUT], f32)
            o16v = o16[:].rearrange("p (c o) -> p c o", o=N_OUT)
            t16 = pool.tile([P, CHUNKS * N_OUT], f32, tag="t16")
            t16v = t16[:].rearrange("p (c o) -> p c o", o=N_OUT)
            for f in range(6):
                a_b = accv[:, :, f:f + 1].to_broadcast([P, CHUNKS, N_OUT])
                w_b = wrv[:, f:f + 1, :].to_broadcast([P, CHUNKS, N_OUT])
                if f == 0:
                    nc.vector.tensor_tensor(o16v, a_b, w_b, op=AT.mult)
                else:
                    nc.vector.tensor_tensor(t16v, a_b, w_b, op=AT.mult)
                    nc.vector.tensor_tensor(o16v, o16v, t16v, op=AT.add)
            nc.vector.tensor_scalar_max(o16[:], o16[:], 0.0)

            # out rows r=c*128+p
            nc.sync.dma_start(
                out=bass.AP(out_t, 0, [[N_OUT, P], [P * N_OUT, CHUNKS], [1, N_OUT]]),
                in_=o16v,
            )

    nc.compile()
    return nc


def kernel(**inputs):
    global LAST_EXEC_NS
    atom = inputs["atom"]
    edge_index = inputs["edge_index"]
    W = inputs["W"]
    b = inputs["b"]

    prep = _host_prepare(atom, edge_index, W, b)
    nc = _build_graph(prep["K"], prep["calls"], prep["S_TOT"], prep["M_TOT"])

    from concourse import bass_utils

    w_rep = np.ascontiguousarray(
        np.tile(prep["W_ext"].T.reshape(1, 6 * N_OUT), (P, 1)))
    in_maps = []
    for ci in range(N_CORES):
        in_maps.append({
            "atom_pack": prep["atom_pack"],
            "degn": prep["degn"],
            "deg_pi": prep["deg_pi"][ci],
            "atom_pi": prep["atom_pi"][ci],
            "w_rep": w_rep,
            "idx_all": _wrap16(prep["idx_feeds"][ci]),
            "mask_all": prep["mask_feeds"][ci],
        })

    trace = bool(os.environ.get("KERNEL_TRACE"))
    if trace:
        try:
            import tracing_shim
            tracing_shim.install()
        except Exception:
            trace = False

    res = bass_utils.run_bass_kernel_spmd(
        nc, in_maps, core_ids=list(range(N_CORES)), trace=trace
    )
    LAST_EXEC_NS = res.exec_time_ns
    globals()["LAST_RES"] = res

    out = np.empty((N_NODES, N_OUT), np.float32)
    for ci in range(N_CORES):
        rows = res.results[ci]["out"]  # [NPC_PAD, 16], row j -> node pi[j]
        pic = prep["pi"][ci]
        real = pic < NPC
        out[ci * NPC + pic[real]] = rows[real]
    return out


# revision 7
# speedup vs baseline: 17.6067x; 15.5674x over previous
"""AtomConv (GCN message passing) distributed Bass kernel for 8 TRN2 NeuronCores.

out = relu(D^-1/2 (A+I) D^-1/2 (atom @ W.T + b)),  A = 3.2M random edges over 100K nodes.

Sharding (per the dst-routing hint): nodes 12500/core, edges routed to the core
owning the destination, weights replicated. Aggregation runs in 6-dim input
space: z[s] = [atom[s]*dis[s], dis[s]]; agg[d] = sum_{s->d} z[s];
out[d] = relu((dis[d]*agg[d]) @ [W|b].T).

Device mechanism: one global z-table [25088 rows x 256B] where row k packs the
z-vectors of nodes 4k..4k+3 at 6-f32 pitch (cols 24:64 zero).  The per-edge
gather uses gpsimd dma_gather (256B elements, int16 row idx = src//4); a
host-fed one-hot mask [slot, 4] selects the wanted sub-row on the DVE
(mask 0 for padding slots, so no zero-row/idx+1 tricks are needed).  Slots
form a single degree-sorted grid (128 dst rows/chunk x K[c] cols, K maxed
across cores so one SPMD graph serves all 8 cores); each chunk's masked slots
reduce directly into the accumulator (no cross-quarter combines).  A DVE
matvec (6->16) + relu finishes on device.  Host work is routing/layout only
(bincount, sort, index/mask packing) plus the final row unpermute/concat.
"""

import os
import numpy as np

N_NODES = 100000
N_IN = 5
N_OUT = 16
N_CORES = 8
NPC = N_NODES // N_CORES            # 12500
P = 128
NPC_PAD = ((NPC + P - 1) // P) * P  # 12544
CHUNKS = NPC_PAD // P               # 98
RPN = 4                             # nodes packed per 256B table row
TA = 196                            # table "a" dim: rows k = a*128+p
TROWS = TA * P                      # 25088 table rows >= 100000/4
ES = 64                             # table row = 64 f32 = 256B
MAX_CALL = 8192                     # slots per dma_gather call

LAST_EXEC_NS = None


def _host_prepare(atom, edge_index, W, b):
    src = np.asarray(edge_index[0]).astype(np.int64)
    dst = np.asarray(edge_index[1]).astype(np.int64)
    # deg includes the self loop; self-loop messages are added directly on
    # device (no gather slot needed)
    deg = (np.bincount(dst, minlength=N_NODES) + 1.0).astype(np.float32)

    core_of = dst // NPC

    # per-core in-core dst degree (real edges only) -> degree-sorted grid
    cnt = np.zeros((N_CORES, NPC_PAD), np.int64)
    per = {}
    for ci in range(N_CORES):
        mc = core_of == ci
        per[ci] = (dst[mc] - ci * NPC, src[mc])
        cnt[ci, :NPC] = np.bincount(per[ci][0], minlength=NPC)
    pi = np.argsort(cnt, axis=1, kind="stable")          # ascending degree
    cnt_sorted = np.take_along_axis(cnt, pi, axis=1)
    K = cnt_sorted.reshape(N_CORES, CHUNKS, P).max(axis=2).max(axis=0)
    K = np.maximum(K, 1).astype(np.int64)                # [CHUNKS] template

    # call plan: whole chunks greedily packed into <= MAX_CALL slots
    calls, cur, cur_slots = [], [], 0
    for c in range(CHUNKS):
        s = int(K[c]) * P
        if cur_slots + s > MAX_CALL and cur:
            calls.append(cur)
            cur, cur_slots = [], 0
        cur.append((c, int(K[c]), cur_slots // P))       # (chunk, K, col offset)
        cur_slots += s
    if cur:
        calls.append(cur)
    S_TOT = int(K.sum()) * P
    M_TOT = S_TOT // P

    Kmax = int(K.max())
    idx_feeds, mask_feeds = [], []
    for ci in range(N_CORES):
        d_loc, s_glob = per[ci]
        order = np.argsort(d_loc, kind="stable")
        d_s, s_s = d_loc[order], s_glob[order]
        starts = np.zeros(NPC, np.int64)
        starts[1:] = np.cumsum(cnt[ci, :NPC])[:-1]
        kk = np.arange(len(d_s)) - starts[d_s]
        mat_idx = np.zeros((NPC_PAD, Kmax), np.int16)
        mat_sub = np.zeros((NPC_PAD, Kmax), np.int8)
        mat_val = np.zeros((NPC_PAD, Kmax), bool)
        mat_idx[d_s, kk] = (s_s // RPN).astype(np.int16)
        mat_sub[d_s, kk] = (s_s % RPN).astype(np.int8)
        mat_val[d_s, kk] = True
        g_idx = mat_idx[pi[ci]]
        g_sub = mat_sub[pi[ci]]
        g_val = mat_val[pi[ci]]
        idx_parts, mask_parts = [], []
        for call in calls:
            for (c, kc, _) in call:
                rows = slice(c * P, (c + 1) * P)
                idx_parts.append(g_idx[rows, :kc].T.reshape(-1))
                sub = g_sub[rows, :kc].T.reshape(-1)
                val = g_val[rows, :kc].T.reshape(-1)
                m = np.zeros((len(sub), RPN), np.float32)
                m[np.arange(len(sub)), sub] = val.astype(np.float32)
                mask_parts.append(m)
        idx_feeds.append(np.concatenate(idx_parts))
        mflat = np.concatenate(mask_parts)               # [S_TOT, 4] slot-major
        mask_feeds.append(np.ascontiguousarray(
            mflat.reshape(M_TOT, P, RPN).transpose(1, 0, 2).reshape(P, M_TOT * RPN)))

    # atom packed in table layout: node n = 4*(a*128+p)+j at (p, a, j)
    nid = (RPN * (np.arange(TA)[None, :, None] * P + np.arange(P)[:, None, None])
           + np.arange(RPN)[None, None, :])              # [P, TA, RPN]
    valid = nid < N_NODES
    nsafe = np.where(valid, nid, 0)
    a_np = np.asarray(atom, np.float32)
    atom_pack = np.zeros((P, TA, RPN, 6), np.float32)
    atom_pack[:, :, :, :N_IN] = a_np[nsafe] * valid[..., None]
    atom_pack[:, :, :, N_IN] = 1.0
    degn = np.where(valid, deg[nsafe], 1.0).astype(np.float32)  # [P, TA, RPN]

    # dst-side (pi-ordered) feeds: row r=c*128+p -> node pi[r]
    deg_pi, atom_pi = [], []
    for ci in range(N_CORES):
        dpc = np.ones(NPC_PAD, np.float32)
        dpc[:NPC] = deg[ci * NPC:(ci + 1) * NPC]
        dpc = np.maximum(dpc[pi[ci]], 1.0)
        deg_pi.append(np.ascontiguousarray(dpc.reshape(CHUNKS, P).T))  # [P, CH]
        apc = np.zeros((NPC_PAD, 6), np.float32)
        apc[:NPC, :N_IN] = a_np[ci * NPC:(ci + 1) * NPC]
        apc[:, N_IN] = 1.0
        apc = apc[pi[ci]].reshape(CHUNKS, P, 6)
        atom_pi.append(np.ascontiguousarray(apc.transpose(1, 0, 2)))   # [P, CH, 6]

    W_ext = np.zeros((N_OUT, 6), np.float32)
    W_ext[:, :N_IN] = np.asarray(W, np.float32)
    W_ext[:, N_IN] = np.asarray(b, np.float32)

    return dict(K=K, pi=pi, calls=calls, S_TOT=S_TOT, M_TOT=M_TOT,
                idx_feeds=idx_feeds, mask_feeds=mask_feeds,
                atom_pack=atom_pack.reshape(P, TA * RPN * 6), degn=degn.reshape(P, TA * RPN),
                deg_pi=deg_pi, atom_pi=atom_pi, W_ext=W_ext)


def _wrap16(flat):
    """idx j -> sbuf (j%16, j//16), replicated across the 8 q7 cores."""
    n = len(flat)
    w = flat.reshape(n // 16, 16).T
    return np.ascontiguousarray(np.tile(w, (8, 1)).astype(np.int16))


def _build_graph(K, calls, S_TOT, M_TOT):
    import concourse.bass as bass
    import concourse.bacc as bacc
    import concourse.mybir as mybir
    import concourse.tile as tile
    from concourse import library_config

    f32 = mybir.dt.float32
    i16 = mybir.dt.int16
    AT = mybir.AluOpType
    AX = mybir.AxisListType

    S_call_max = max(sum(kc for (_, kc, _) in call) for call in calls) * P
    M_call_max = S_call_max // P

    nc = bacc.Bacc("TRN2", target_bir_lowering=False, debug=False,
                   num_swdge_queues=4)

    atom_in = nc.dram_tensor("atom_pack", [P, TA * RPN * 6], f32, kind="ExternalInput")
    degn_in = nc.dram_tensor("degn", [P, TA * RPN], f32, kind="ExternalInput")
    degp_in = nc.dram_tensor("deg_pi", [P, CHUNKS], f32, kind="ExternalInput")
    atomp_in = nc.dram_tensor("atom_pi", [P, CHUNKS, 6], f32, kind="ExternalInput")
    wrep_in = nc.dram_tensor("w_rep", [P, 6 * N_OUT], f32, kind="ExternalInput")
    idx_in = nc.dram_tensor("idx_all", [P, S_TOT // 16], i16, kind="ExternalInput")
    mask_in = nc.dram_tensor("mask_all", [P, M_TOT * RPN], f32, kind="ExternalInput")
    out_t = nc.dram_tensor("out", [NPC_PAD, N_OUT], f32, kind="ExternalOutput")

    z_dram = nc.dram_tensor("z_tab", [TROWS, ES], f32, kind="Internal")

    with tile.TileContext(nc) as tc:
        with tc.tile_pool(name="sb", bufs=1) as pool, \
             tc.tile_pool(name="zt", bufs=2) as ztpool, \
             tc.tile_pool(name="gp", bufs=4) as gpool, \
             tc.tile_pool(name="tp", bufs=3) as tpool, \
             tc.tile_pool(name="ip", bufs=4) as ipool, \
             tc.tile_pool(name="mp", bufs=4) as mpool:
            nc.gpsimd.load_library(library_config.mlp)

            # ---- source-side dis and packed z-table build (4 pipelined pieces)
            at = pool.tile([P, TA * RPN * 6], f32, tag="at")
            dgn = pool.tile([P, TA * RPN], f32, tag="dgn")
            dsn = pool.tile([P, TA * RPN], f32, tag="dsn")
            nc.sync.dma_start(out=at[:], in_=atom_in.ap())
            nc.sync.dma_start(out=dgn[:], in_=degn_in.ap())
            nc.vector.reciprocal(dsn[:], dgn[:])
            nc.scalar.activation(dsn[:], dsn[:], mybir.ActivationFunctionType.Sqrt)

            atv = at[:].rearrange("p (a j f) -> p a j f", j=RPN, f=6)
            dsv = dsn[:].rearrange("p (a j f) -> p a j f", j=RPN, f=1)
            HA = TA // 4
            for piece in range(4):
                zt = ztpool.tile([P, HA * ES], f32, tag="zt")
                nc.vector.memset(zt[:], 0.0)
                ztv = zt[:].rearrange("p (a e) -> p a e", e=ES)
                ztj = ztv[:, :, 0:RPN * 6].rearrange("p a (j f) -> p a j f", f=6)
                sl = slice(piece * HA, (piece + 1) * HA)
                nc.vector.tensor_tensor(
                    ztj, atv[:, sl], dsv[:, sl].to_broadcast([P, HA, RPN, 6]),
                    op=AT.mult)
                nc.sync.dma_start(
                    out=bass.AP(z_dram, piece * HA * P * ES,
                                [[ES, P], [P * ES, HA], [1, ES]]),
                    in_=zt[:],
                )

            # ---- dst-side dis and self-loop term (no gather deps)
            dgp = pool.tile([P, CHUNKS], f32, tag="dgp")
            dsp = pool.tile([P, CHUNKS], f32, tag="dsp")
            nc.sync.dma_start(out=dgp[:], in_=degp_in.ap())
            nc.vector.reciprocal(dsp[:], dgp[:])
            nc.scalar.activation(dsp[:], dsp[:], mybir.ActivationFunctionType.Sqrt)
            ap0 = pool.tile([P, CHUNKS * 6], f32, tag="ap0")
            nc.sync.dma_start(out=ap0[:], in_=atomp_in.ap().rearrange("p c f -> p (c f)"))
            sl6 = pool.tile([P, CHUNKS * 6], f32, tag="sl6")
            sl6v = sl6[:].rearrange("p (c f) -> p c f", f=6)
            dspv = dsp[:].rearrange("p (c f) -> p c f", f=1)
            nc.vector.tensor_tensor(
                sl6v, ap0[:].rearrange("p (c f) -> p c f", f=6),
                dspv.to_broadcast([P, CHUNKS, 6]), op=AT.mult)

            acc = pool.tile([P, CHUNKS * 6], f32)
            accv = acc[:].rearrange("p (c f) -> p c f", f=6)

            wr = pool.tile([P, 6 * N_OUT], f32, tag="wr")
            nc.sync.dma_start(out=wr[:], in_=wrep_in.ap())
            wrv = wr[:].rearrange("p (f o) -> p f o", o=N_OUT)
            o16 = pool.tile([P, CHUNKS * N_OUT], f32)
            o16v = o16[:].rearrange("p (c o) -> p c o", o=N_OUT)
            t16 = pool.tile([P, CHUNKS * N_OUT], f32, tag="t16")
            t16v = t16[:].rearrange("p (c o) -> p c o", o=N_OUT)

            def finish(lo, hi):
                # add self-loop term, dis_dst scale, 6->16 matvec, relu, out
                n = hi - lo
                nc.vector.tensor_tensor(acc[:, lo * 6:hi * 6], acc[:, lo * 6:hi * 6],
                                        sl6[:, lo * 6:hi * 6], op=AT.add)
                nc.vector.tensor_tensor(
                    accv[:, lo:hi, :], accv[:, lo:hi, :],
                    dspv[:, lo:hi].to_broadcast([P, n, 6]), op=AT.mult)
                for f in range(6):
                    a_b = accv[:, lo:hi, f:f + 1].to_broadcast([P, n, N_OUT])
                    w_b = wrv[:, f:f + 1, :].to_broadcast([P, n, N_OUT])
                    if f == 0:
                        nc.vector.tensor_tensor(o16v[:, lo:hi, :], a_b, w_b, op=AT.mult)
                    else:
                        nc.vector.tensor_tensor(t16v[:, lo:hi, :], a_b, w_b, op=AT.mult)
                        nc.vector.tensor_tensor(o16v[:, lo:hi, :], o16v[:, lo:hi, :],
                                                t16v[:, lo:hi, :], op=AT.add)
                nc.vector.tensor_scalar_max(o16[:, lo * N_OUT:hi * N_OUT],
                                            o16[:, lo * N_OUT:hi * N_OUT], 0.0)
                nc.sync.dma_start(
                    out=bass.AP(out_t, lo * P * N_OUT,
                                [[N_OUT, P], [P * N_OUT, n], [1, N_OUT]]),
                    in_=o16v[:, lo:hi, :],
                )

            split_chunk = calls[-1][0][0] if len(calls) > 1 else 0

            # ---- gather + masked reduce per chunk
            qn = 0
            off = 0
            for ci_call, call in enumerate(calls):
                S = sum(kc for (_, kc, _) in call) * P
                M = S // P
                it = ipool.tile([P, S_call_max // 16], i16, tag="idx")
                nc.sync.dma_start(
                    out=it[:, : S // 16],
                    in_=idx_in[:, off // 16:(off + S) // 16])
                mt = mpool.tile([P, M_call_max * RPN], f32, tag="msk")
                nc.sync.dma_start(
                    out=mt[:, : M * RPN],
                    in_=mask_in[:, (off // P) * RPN:(off // P + M) * RPN])
                gb = gpool.tile([P, M_call_max * ES], f32, tag="gb")
                gbv = gb[:].rearrange("p (m e) -> p m e", m=M_call_max)
                nc.gpsimd.dma_gather(
                    out_ap=gbv[:, :M, :],
                    in_ap=z_dram.ap(),
                    idxs_ap=it[:, : S // 16],
                    num_idxs=S,
                    num_idxs_reg=S,
                    elem_size=ES,
                    single_packet=False,
                    queue_num=qn % 4,
                )
                qn += 1
                gb24 = gbv[:, :M, 0:RPN * 6].rearrange("p m (j f) -> p m j f", f=6)
                tmp = tpool.tile([P, M_call_max * RPN * 6], f32, tag="tmp")
                tmpv = tmp[:, : M * RPN * 6].rearrange("p (m j f) -> p m j f", j=RPN, f=6)
                mtv = mt[:, : M * RPN].rearrange("p (m j f) -> p m j f", j=RPN, f=1)
                nc.vector.tensor_tensor(
                    tmpv, gb24, mtv.to_broadcast([P, M, RPN, 6]), op=AT.mult)
                # merge consecutive equal-K chunks into one 4d reduce
                runs, i = [], 0
                while i < len(call):
                    j = i
                    while j + 1 < len(call) and call[j + 1][1] == call[i][1]:
                        j += 1
                    runs.append((call[i][0], j - i + 1, call[i][1], call[i][2]))
                    i = j + 1
                for (c0, nch, kc, colofs) in runs:
                    w = kc * RPN * 6
                    seg = tmp[:, colofs * RPN * 6:colofs * RPN * 6 + nch * w] \
                        .rearrange("p (c k f) -> p c f k", k=kc * RPN, f=6)
                    nc.vector.tensor_reduce(accv[:, c0:c0 + nch, :], seg,
                                            axis=AX.X, op=AT.add)
                off += S
                if ci_call == len(calls) - 2 and split_chunk > 0:
                    finish(0, split_chunk)

            if split_chunk > 0:
                finish(split_chunk, CHUNKS)
            else:
                finish(0, CHUNKS)

    nc.compile()
    return nc


def kernel(**inputs):
    global LAST_EXEC_NS
    atom = inputs["atom"]
    edge_index = inputs["edge_index"]
    W = inputs["W"]
    b = inputs["b"]

    prep = _host_prepare(atom, edge_index, W, b)
    nc = _build_graph(prep["K"], prep["calls"], prep["S_TOT"], prep["M_TOT"])

    from concourse import bass_utils

    w_rep = np.ascontiguousarray(
        np.tile(prep["W_ext"].T.reshape(1, 6 * N_OUT), (P, 1)))
    in_maps = []
    for ci in range(N_CORES):
        in_maps.append({
            "atom_pack": prep["atom_pack"],
            "degn": prep["degn"],
            "deg_pi": prep["deg_pi"][ci],
            "atom_pi": prep["atom_pi"][ci],
            "w_rep": w_rep,
            "idx_all": _wrap16(prep["idx_feeds"][ci]),
            "mask_all": prep["mask_feeds"][ci],
        })

    trace = bool(os.environ.get("KERNEL_TRACE"))
    if trace:
        try:
            import tracing_shim
            tracing_shim.install()
        except Exception:
            trace = False

    res = bass_utils.run_bass_kernel_spmd(
        nc, in_maps, core_ids=list(range(N_CORES)), trace=trace
    )
    LAST_EXEC_NS = res.exec_time_ns
    globals()["LAST_RES"] = res

    out = np.empty((N_NODES, N_OUT), np.float32)
    for ci in range(N_CORES):
        rows = res.results[ci]["out"]  # [NPC_PAD, 16], row j -> node pi[j]
        pic = prep["pi"][ci]
        real = pic < NPC
        out[ci * NPC + pic[real]] = rows[real]
    return out
